# revision 1
# baseline (speedup 1.0000x reference)
"""ACmix Trainium2 kernel: batch-parallel over 8 NeuronCores.

Per-core graph (one batch element, x: (256, 64, 64)):
  - qkv 1x1 conv as matmul, with the grouped kernel-generator folded in on the
    host (W_kgx = w_kg_g @ w_qkv_grouped) so it rides along as 36 extra M rows.
  - 3x3 window shifts are free-axis AP offsets into zero-padded (66-pitch)
    k/v buffers.
  - logits: DVE/GPSIMD shifted elementwise products + PE "selector" matmuls
    (constant block one-hot lhsT) that reduce over d and land compact
    (18, 512) tiles in PSUM.
  - softmax without max-subtraction (logits bounded ~|5|), exp on ScalarE,
    Z via PE replication matmul, reciprocal on DVE.
  - combined coefficients Wc = alpha*kernel + beta*attn (one field for both
    branches), V-contraction via PE replication matmuls + DVE multiply +
    DVE/GPSIMD accumulation.
  - final 1x1 proj matmul, DMA out f32.
"""

import os
import sys

import numpy as np

sys.path.insert(0, "/opt/trn_rl_repo")

H_IMG = 64
W_IMG = 64
L = H_IMG * W_IMG  # 4096
C = 256
NH = 4
D = 64
K2 = 9
PITCH = 66
PADLEN = PITCH * PITCH  # 4356
NCORES = 8
NT = 8  # 512-column chunks
ROWS_PER_NT = 8  # image rows per 512 chunk

LAST_EXEC_NS = None
LAST_TRACE_DIR = None


def _build_graph(alpha, beta, phases="BKCDEF"):
    import concourse.bacc as bacc
    import concourse.bass as bass
    import concourse.mybir as mybir
    from concourse import tile

    f32 = mybir.dt.float32
    bf16 = mybir.dt.bfloat16
    MULT = mybir.AluOpType.mult
    ADD = mybir.AluOpType.add
    EXP = mybir.ActivationFunctionType.Exp
    COPY = mybir.ActivationFunctionType.Copy

    nc = bacc.Bacc(None, target_bir_lowering=False)

    x_ext = nc.declare_dram_parameter("x", [C, L], f32, isOutput=False)
    wt_ext = nc.declare_dram_parameter("wt", [C, 804], f32, isOutput=False)
    wpt_ext = nc.declare_dram_parameter("wpt", [C, C], f32, isOutput=False)
    osel_ext = nc.declare_dram_parameter("osel", [128, 9 * 18], f32, isOutput=False)
    orep_ext = nc.declare_dram_parameter("orep", [18, 9 * 128], f32, isOutput=False)
    ozrep_ext = nc.declare_dram_parameter("ozrep", [18, 18], f32, isOutput=False)
    ident_ext = nc.declare_dram_parameter("ident", [128, 128], f32, isOutput=False)
    bkg0_ext = nc.declare_dram_parameter("bkg0", [18, 1], f32, isOutput=False)
    bkg1_ext = nc.declare_dram_parameter("bkg1", [18, 1], f32, isOutput=False)
    out_ext = nc.declare_dram_parameter("out", [C, L], f32, isOutput=True)

    with tile.TileContext(nc) as tc:
        with (
            tc.tile_pool(name="const", bufs=1) as cpool,
            tc.tile_pool(name="data", bufs=1) as dpool,
            tc.tile_pool(name="prod", bufs=4) as ppool,
            tc.tile_pool(name="pv", bufs=4) as pvpool,
            tc.tile_pool(name="outsb", bufs=3) as opool,
            tc.tile_pool(name="ps_mm", bufs=2, space="PSUM") as ps_mm,
            tc.tile_pool(name="ps_lg", bufs=2, space="PSUM") as ps_lg,
            tc.tile_pool(name="ps_rep", bufs=3, space="PSUM") as ps_rep,
            tc.tile_pool(name="ps_acc", bufs=1, space="PSUM") as ps_acc,
        ):
            # ---- constants ----
            wt_bf = [cpool.tile([128, 804], bf16, tag=f"wt{k}", name=f"wt{k}") for k in range(2)]
            wpt_bf = [cpool.tile([128, C], bf16, tag=f"wpt{k}", name=f"wpt{k}") for k in range(2)]
            osel_bf = cpool.tile([128, 9 * 18], bf16, tag="osel")
            orep_bf = cpool.tile([18, 9 * 128], bf16, tag="orep")
            ozrep_bf = cpool.tile([18, 18], bf16, tag="ozrep")
            ident_bf = cpool.tile([128, 128], bf16, tag="ident")
            bkg_sb = [cpool.tile([18, 1], f32, tag=f"bkg{k}", name=f"bkg{k}") for k in range(2)]
            for k in range(2):
                nc.gpsimd.dma_start(wt_bf[k][:], wt_ext[128 * k : 128 * (k + 1), :])
                nc.gpsimd.dma_start(wpt_bf[k][:], wpt_ext[128 * k : 128 * (k + 1), :])
            nc.gpsimd.dma_start(osel_bf[:], osel_ext[:])
            nc.gpsimd.dma_start(orep_bf[:], orep_ext[:])
            nc.gpsimd.dma_start(ozrep_bf[:], ozrep_ext[:])
            nc.gpsimd.dma_start(ident_bf[:], ident_ext[:])
            nc.gpsimd.dma_start(bkg_sb[0][:], bkg0_ext[:])
            nc.gpsimd.dma_start(bkg_sb[1][:], bkg1_ext[:])

            # ---- input (cast f32 -> bf16 during DMA) ----
            x_bf = [dpool.tile([128, L], bf16, tag=f"x{k}", name=f"x{k}") for k in range(2)]
            for k in range(2):
                nc.gpsimd.dma_start(x_bf[k][:], x_ext[128 * k : 128 * (k + 1), :])

            # ---- main SBUF tensors ----
            q_bf = [dpool.tile([128, L], bf16, tag=f"q{h}", name=f"q{h}") for h in range(2)]
            GUARD = 66
            FLATW = GUARD + L + GUARD  # 4228
            ke = [dpool.tile([128, FLATW], bf16, tag=f"ke{h}", name=f"ke{h}") for h in range(2)]
            ko = [dpool.tile([128, FLATW], bf16, tag=f"ko{h}", name=f"ko{h}") for h in range(2)]
            ve = [dpool.tile([128, FLATW], bf16, tag=f"ve{h}", name=f"ve{h}") for h in range(2)]
            kg = [dpool.tile([18, L], bf16, tag=f"kg{h}", name=f"kg{h}") for h in range(2)]
            expS = [dpool.tile([18, L], bf16, tag=f"expS{h}", name=f"expS{h}") for h in range(2)]
            wc = [dpool.tile([18, L], bf16, tag=f"wc{h}", name=f"wc{h}") for h in range(2)]
            lnz = [dpool.tile([18, L], bf16, tag=f"lnz{h}", name=f"lnz{h}") for h in range(2)]
            invz = [dpool.tile([18, L], bf16, tag=f"invz{h}", name=f"invz{h}") for h in range(2)]
            acc = [dpool.tile([128, L], bf16, tag=f"acc{h}", name=f"acc{h}") for h in range(2)]

            # zero the guard bands
            for t in ke + ve:
                nc.vector.memset(t[:, 0:GUARD], 0.0)
                nc.vector.memset(t[:, GUARD + L : FLATW], 0.0)

            # ---- phase B: qkv + kernel-gen matmul, evictions ----
            # M tiles: 6x128 (qkv) then 2x18 (kernel rows)
            m_tiles = [(m * 128, 128) for m in range(6)] + [(768, 18), (786, 18)]
            for nt in range(NT):
                ncol = slice(nt * 512, (nt + 1) * 512)
                for mi, (m0, msz) in enumerate(m_tiles):
                    ps = ps_mm.tile([msz, 512], f32, tag="mmps", name="qkvps", padded_shape=[128, 512])
                    for kt in range(2):
                        nc.tensor.matmul(
                            ps[:],
                            wt_bf[kt][:, m0 : m0 + msz],
                            x_bf[kt][:, ncol],
                            start=(kt == 0),
                            stop=(kt == 1),
                        )
                    if mi < 2:
                        # q rows -> q_bf[mi]
                        nc.vector.tensor_copy(q_bf[mi][:, ncol], ps[:])
                    elif mi < 4:
                        hp = mi - 2
                        dst = ke[hp][:, GUARD + 512 * nt : GUARD + 512 * (nt + 1)]
                        nc.vector.tensor_copy(dst, ps[:])
                    elif mi < 6:
                        hp = mi - 4
                        dst = ve[hp][:, GUARD + 512 * nt : GUARD + 512 * (nt + 1)]
                        nc.scalar.activation(dst, ps[:], COPY)
                    else:
                        hp = mi - 6
                        nc.scalar.activation(
                            kg[hp][:, ncol],
                            ps[:],
                            mybir.ActivationFunctionType.Identity,
                            bias=bkg_sb[hp][:],
                        )

            # odd-offset copies of k (ko[:, i+1] = ke[:, i]) for DVE 2x alignment
            for hp in range(2):
                nc.vector.tensor_copy(ko[hp][:, 1:FLATW], ke[hp][:, 0 : FLATW - 1])
                nc.vector.memset(ko[hp][:, 0:1], 0.0)

            # shift table: s = 3*(di+1) + (dj+1), flat offset 64*di + dj
            shifts = []
            for di in (-1, 0, 1):
                for dj in (-1, 0, 1):
                    shifts.append((di, dj))

            def k_view(hp, s, l0, ncols):
                di, dj = shifts[s]
                off = 64 * di + dj
                if (GUARD + off) % 2 == 0:
                    base = GUARD + off + l0
                    return ke[hp][:, base : base + ncols]
                base = GUARD + 1 + off + l0
                return ko[hp][:, base : base + ncols]

            def v_view(hp, s, l0, ncols):
                di, dj = shifts[s]
                off = 64 * di + dj
                base = GUARD + off + l0
                return ve[hp][:, base : base + ncols]

            def zero_edge(eng, t, dj, ncols):
                # zero wrapped image-columns: dj=-1 -> j=0, dj=+1 -> j=63
                if dj == 0:
                    return
                v3 = t.rearrange("p (r c) -> p r c", c=64)
                j = 0 if dj == -1 else 63
                eng.memset(v3[:, :, j : j + 1], 0.0)

            # ---- phase C: logits (products + selector matmul) + exp ----
            for hp in range(2 if "C" in phases else 0):
                for g in range(4):  # 1024-wide groups
                    gc = slice(g * 1024, (g + 1) * 1024)
                    r0 = 16 * g
                    qv = q_bf[hp][:, gc]
                    lgs = [
                        ps_lg.tile([18, 512], f32, tag="lg", name="lg")
                        for _ in range(2)
                    ]
                    for s in range(K2):
                        prod = ppool.tile([128, 1024], bf16, tag="prod", name="prod")
                        kap = k_view(hp, s, g * 1024, 1024)
                        nc.vector.tensor_tensor(prod[:], qv, kap, MULT)
                        zero_edge(nc.gpsimd, prod[:], shifts[s][1], 1024)
                        for j in range(2):
                            nc.tensor.matmul(
                                lgs[j][:],
                                osel_bf[:, s * 18 : (s + 1) * 18],
                                prod[:, j * 512 : (j + 1) * 512],
                                start=(s == 0),
                                stop=(s == K2 - 1),
                            )
                    for j in range(2):
                        ncol = slice(g * 1024 + j * 512, g * 1024 + (j + 1) * 512)
                        nc.scalar.activation(expS[hp][:, ncol], lgs[j][:], EXP)

            # ---- phase D: Z, invZ = exp(-ln Z), combined coefficients ----
            LOG = mybir.ActivationFunctionType.Ln
            for hp in range(2 if "D" in phases else 0):
                for nt in range(NT):
                    ncol = slice(nt * 512, (nt + 1) * 512)
                    zr = ps_lg.tile([18, 512], f32, tag="lg", name="zr")
                    nc.tensor.matmul(zr[:], ozrep_bf[:], expS[hp][:, ncol])
                    nc.scalar.activation(lnz[hp][:, ncol], zr[:], LOG)
                nc.scalar.activation(invz[hp][:], lnz[hp][:], EXP, scale=-1.0)
                # wc = (invz * beta) * expS + alpha * kg  (two fused wide ops)
                for h2 in range(2):
                    hc = slice(h2 * 2048, (h2 + 1) * 2048)
                    nc.vector.scalar_tensor_tensor(
                        wc[hp][:, hc], invz[hp][:, hc], float(beta), expS[hp][:, hc], MULT, MULT
                    )
                    nc.vector.scalar_tensor_tensor(
                        wc[hp][:, hc], kg[hp][:, hc], float(alpha), wc[hp][:, hc], MULT, ADD
                    )

            # ---- phase E: V-contraction (PE accumulates; rep lookahead to
            # avoid PE FIFO head-of-line blocking on the DVE mults) ----
            for hp in range(2 if "E" in phases else 0):
                for g in range(NT):  # 512-wide groups
                    gc = slice(g * 512, (g + 1) * 512)
                    aps = ps_acc.tile([128, 512], f32, tag="accps", name="accps")
                    rps = {}
                    pvs = {}

                    def emit_rep(s):
                        rp = ps_rep.tile([128, 512], f32, tag="rep", name="rep")
                        nc.tensor.matmul(
                            rp[:],
                            orep_bf[:, s * 128 : (s + 1) * 128],
                            wc[hp][:, g * 512 : (g + 1) * 512],
                        )
                        rps[s] = rp

                    def emit_mult(s):
                        pv = pvpool.tile([128, 512], bf16, tag="pv", name="pv")
                        nc.vector.tensor_tensor(
                            pv[:], rps[s][:], v_view(hp, s, g * 512, 512), MULT
                        )
                        zero_edge(nc.gpsimd, pv[:], shifts[s][1], 512)
                        pvs[s] = pv

                    emit_rep(0)
                    emit_mult(0)
                    emit_rep(1)
                    emit_mult(1)
                    for s in range(K2):
                        if s + 2 < K2:
                            emit_rep(s + 2)
                            emit_mult(s + 2)
                        nc.tensor.matmul(
                            aps[:],
                            ident_bf[:],
                            pvs[s][:],
                            start=(s == 0),
                            stop=(s == K2 - 1),
                        )
                    nc.scalar.activation(acc[hp][:, gc], aps[:], COPY)

            # ---- phase F: proj + DMA out ----
            for mt in range(2 if "F" in phases else 0):
                for nt in range(NT):
                    ncol = slice(nt * 512, (nt + 1) * 512)
                    ps = ps_mm.tile([128, 512], f32, tag="mmps", name="projps")
                    for kt in range(2):
                        nc.tensor.matmul(
                            ps[:],
                            wpt_bf[kt][:, mt * 128 : (mt + 1) * 128],
                            acc[kt][:, ncol],
                            start=(kt == 0),
                            stop=(kt == 1),
                        )
                    ob = opool.tile([128, 512], f32, tag="ob", name="ob")
                    nc.scalar.activation(ob[:], ps[:], COPY)
                    nc.sync.dma_start(
                        out_ext[mt * 128 : (mt + 1) * 128, ncol], ob[:]
                    )

    return nc


def kernel(**inputs):
    global LAST_EXEC_NS, LAST_TRACE_DIR
    x = np.asarray(inputs["x"], np.float32)
    w_qkv = np.asarray(inputs["w_qkv"], np.float32)
    w_kg = np.asarray(inputs["w_kg"], np.float32)
    b_kg = np.asarray(inputs["b_kg"], np.float32).reshape(-1)
    alpha = float(np.asarray(inputs["alpha"]))
    beta = float(np.asarray(inputs["beta"]))
    w_proj = np.asarray(inputs["w_proj"], np.float32)

    B = x.shape[0]
    # fold grouped kernel-generator through qkv weights
    W_kgx = np.zeros((NH * K2, C), np.float32)
    for h in range(NH):
        W_kgx[h * K2 : (h + 1) * K2] = (
            w_kg[h * K2 : (h + 1) * K2] @ w_qkv[192 * h : 192 * (h + 1)]
        )
    w_aug = np.concatenate([w_qkv, W_kgx], 0)  # (804, 256)
    wt = np.ascontiguousarray(w_aug.T)
    wpt = np.ascontiguousarray(w_proj.T)

    osel = np.zeros((128, 9 * 18), np.float32)
    for s in range(K2):
        for hl in range(2):
            for d in range(D):
                osel[hl * D + d, s * 18 + hl * K2 + s] = 1.0
    orep = np.zeros((18, 9 * 128), np.float32)
    for s in range(K2):
        for hl in range(2):
            orep[hl * K2 + s, s * 128 + hl * D : s * 128 + (hl + 1) * D] = 1.0
    ozrep = np.zeros((18, 18), np.float32)
    for hl in range(2):
        ozrep[hl * K2 : (hl + 1) * K2, hl * K2 : (hl + 1) * K2] = 1.0
    ident = np.eye(128, dtype=np.float32)
    bkg0 = np.ascontiguousarray(b_kg[:18].reshape(18, 1))
    bkg1 = np.ascontiguousarray(b_kg[18:].reshape(18, 1))

    nc = _build_graph(alpha, beta)
    if not nc.is_finalized():
        nc.finalize()

    shared = dict(
        wt=wt, wpt=wpt, osel=osel, orep=orep, ozrep=ozrep, bkg0=bkg0, bkg1=bkg1,
        ident=ident,
    )
    in_maps = [
        dict(shared, x=np.ascontiguousarray(x[b].reshape(C, L))) for b in range(B)
    ]

    from concourse import bass_utils as _bu
    from concourse.bass_utils import run_bass_kernel_spmd

    trace = os.environ.get("KERNEL_TRACE", "0") == "1"
    tkw = {}
    if trace:
        import types

        try:
            import antenv.axon_hooks  # noqa: F401
        except ImportError:
            sys.path.insert(0, "/root/.axon_site")
            from trn_agent_boot.trn_boot import _ntff_profile_via_ctypes

            _mod = types.ModuleType("antenv.axon_hooks")
            _hook = _ntff_profile_via_ctypes("/opt/axon/libaxon_pjrt.so")
            _mod.get_axon_ntff_profile_hook = lambda: _hook
            _mod.set_axon_ntff_profile_hook = lambda h: None
            sys.modules["antenv.axon_hooks"] = _mod
        _bu.upload_artifacts = lambda tmpdir: "local://" + tmpdir
        import tempfile

        global LAST_TRACE_DIR
        LAST_TRACE_DIR = tempfile.mkdtemp(prefix="ktrace_")
        tkw["tmpdir"] = LAST_TRACE_DIR
    res = run_bass_kernel_spmd(
        nc, in_maps, core_ids=list(range(NCORES)), trace=trace, **tkw
    )
    LAST_EXEC_NS = res.exec_time_ns
    out = np.stack(
        [np.asarray(res.results[b]["out"]).reshape(C, H_IMG, W_IMG) for b in range(B)]
    )
    return out.astype(np.float32)

